# revision 10
# baseline (speedup 1.0000x reference)
"""Multi-head cross-attention on 8 Trainium2 NeuronCores.

Problem (hardcoded): input [4, 2048, 1024], memory [4, 2048, 1024],
Wq/Wk/Wv [1024, 1024], bq/bk/bv [1024]; 16 heads x 64 dim; out
[4, 2048, 1024] f32.

Sharding: core c handles batch b = c//2 and head group g = c%2 (8
heads, output columns 512g:512g+512). Embarrassingly parallel - no
collectives.

Device dataflow (per core), everything contracted over channels with
host-pre-transposed operands so no on-chip transposes are needed:
  Q^T[d, n]  = sum_c WqT[c, d] * XT[c, n]     (depth scale folded in WqT)
  K^T[d, m]  = sum_c WkT[c, d] * MT[c, m]
  V[m, d]    = sum_c MT[c, m] * WvT[c, d]
  S^T[m, q]  = sum_d K^T[d, m] * Q^T[d, q]    (per head; heads of a pair
                                               row-packed: tile_position
                                               rows 0-63 / 64-127 run
                                               CONCURRENT on the PE)
  P^T        = exp(S^T)                        (no max subtraction;
                                               logits are O(5), safe)
  outU^T     = V^T P^T                         (per head; the two heads
                                               col-packed: tile_position
                                               cols 0-63 / 64-127 run
                                               CONCURRENT, own pt stream
                                               each -> full PE array)

ScalarE exp (256 ACTIVATEs of [128, 1024], ~1.03us effective each) is
the bottleneck, so the exp + softmax-denominator work is split across
engines per 16-unit (pair, qc) block:
  - mt in SCHRAU_MT: DVE computes exp via the Schraudolph bit trick in
    ONE tensor_scalar: int16(S * 184.66 + B) reinterpreted as bf16.
    ~2-3% elementwise error on those tiles only; the rest stay exact.
  - mt in SHIP_MT: the bf16 exp tile is shipped raw to DRAM (batched
    SWDGE DMA from a contiguous ship buffer); the HOST folds it into
    the softmax denominator. No on-device accumulate for those.
  - mt in GP_MT: GpSimd (idle otherwise) does the accumulate add.
  - remaining mt: DVE accumulates (first two tiles fused into one
    tensor_tensor).
Denominators: host sums the two per-block partial-acc ships (DVE's and
GpSimd's) plus the raw-shipped tiles over their 128 m-partitions, then
divides outU. Biases (zero here) are handled exactly via an extra K=1
contraction chunk when nonzero.

DMA plan: ScalarE's HWDGE ring carries ONLY the Q-projection gate
(per-chunk wq/xt so the first exp isn't delayed); the sync ring carries
the K gate per-chunk then the bulk inputs as few batched 3D DMAs; ALL
output traffic (out blocks, partial accs, raw pt ships) rides gpsimd
SWDGE. Queues fan their descriptors across all 16 DMA engines, so
batched DMAs cost the issuing engine ~1us regardless of size.

PSUM budget: S ping-pong 2x2 banks + proj 2 + pv 2 = 8.
"""

import numpy as np
import ml_dtypes

import concourse.bass as bass
import concourse.mybir as mybir
from concourse import bacc, tile
from concourse.bass_utils import run_bass_kernel_spmd

B, N, M, DIM = 4, 2048, 2048, 1024
NUM_HEADS, HEAD_DIM = 16, 64
HG = 8            # heads per core
COLS = HG * HEAD_DIM  # 512 output cols per core
N_CORES = 8
CC = DIM // 128   # 8 contraction chunks of 128
QC = 4            # q chunks of 512
MC = 16           # m (key) tiles of 128

F32 = mybir.dt.float32
BF16 = mybir.dt.bfloat16
I16 = mybir.dt.int16
EXP = mybir.ActivationFunctionType.Exp
ADD = mybir.AluOpType.add
MULT = mybir.AluOpType.mult

# Per-block (16 mt units) role assignment.
SCHRAU_MT = (2, 8, 14)        # DVE Schraudolph exp (subset of DVE acc set)
SHIP_MT = (1, 4, 7, 10, 13, 15)  # raw-shipped to host, no on-device accumulate
GP_MT = (3, 6, 9, 12)         # GpSimd accumulate pairs (3+6, 9+12)
DVE_ACC_MT = tuple(m for m in range(MC) if m not in SHIP_MT and m not in GP_MT)
N_SHIP = len(SHIP_MT)
SHIP_SLOTS = N_SHIP

# Schraudolph constants: bf16bits(exp(x)) ~= round(x*128*log2(e) + B).
SCHRAU_A = 128.0 * 1.4426950408889634
SCHRAU_B = 127.0 * 128.0 - 4.8

_NC_CACHE = {}
_RUN_KWARGS = {}   # test harness may inject trace=True etc.
LAST_RESULT = None


def _build(with_bias: bool):
    """Build the per-core SPMD Bass program."""
    cc_n = CC + (1 if with_bias else 0)
    nc = bacc.Bacc(None, target_bir_lowering=False)

    xt_ext = nc.declare_dram_parameter("xt", [cc_n, 128, N], BF16, isOutput=False)
    mt_ext = nc.declare_dram_parameter("mt", [cc_n, 128, M], BF16, isOutput=False)
    wq_ext = nc.declare_dram_parameter("wq", [cc_n, 128, COLS], BF16, isOutput=False)
    wk_ext = nc.declare_dram_parameter("wk", [cc_n, 128, COLS], BF16, isOutput=False)
    wv_ext = nc.declare_dram_parameter("wv", [cc_n, 128, COLS], BF16, isOutput=False)
    out_ext = nc.declare_dram_parameter("out", [QC, QC, 128, 512], F32, isOutput=True)
    acc_ext = nc.declare_dram_parameter(
        "accs", [QC, QC, 3, 128, 1024], BF16, isOutput=True)
    raw_ext = nc.declare_dram_parameter(
        "raw", [QC, QC, SHIP_SLOTS, 128, 1024], BF16, isOutput=True)

    ch = [(i, 128) for i in range(CC)]
    if with_bias:
        ch.append((CC, 1))

    with tile.TileContext(nc) as tc:
        with (
            tc.tile_pool(name="acts", bufs=1) as acts,
            tc.tile_pool(name="qkv", bufs=1) as qkv,
            tc.tile_pool(name="pt", bufs=26) as ptp,
            tc.tile_pool(name="daccp", bufs=2) as daccp,
            tc.tile_pool(name="gaccp", bufs=4) as gaccp,
            tc.tile_pool(name="osb", bufs=1) as osb,
            tc.tile_pool(name="ps_s", bufs=2, space="PSUM") as ps_sp,
            tc.tile_pool(name="ps_proj", bufs=2, space="PSUM") as ps_proj,
            tc.tile_pool(name="ps_o", bufs=2, space="PSUM") as ps_op,
        ):
            wk_sb = acts.tile([128, cc_n, COLS], BF16)
            wq_sb = acts.tile([128, cc_n, COLS], BF16)
            xt_sb = acts.tile([128, cc_n, N], BF16)
            wv_sb = acts.tile([128, cc_n, COLS], BF16)
            mt_sb = acts.tile([128, cc_n, M], BF16)

            # --- input DMAs ---
            # Gates per-chunk so projections overlap arrival: K gate on
            # the sync HWDGE ring, Q gate on the scalar ring (its ONLY
            # duty - ScalarE must spend the steady state on exp). Bulk
            # follows on sync as batched 3D DMAs (descriptors fan out
            # across all 16 DMA engines; issue cost ~600ns each).
            # The K00+Q00 gate (wk/wq/mt lo + xt qc0, 2.56MB) paces the
            # first exp: interleave per-chunk on sync so projections track
            # arrival; xt qc0 rides both rings (lo chunks on the scalar
            # ring - its ONLY dma duty). Exp-feeding bulk (mt) outranks wv
            # (PV backlog absorbs V lateness); xt/w bulk rides SWDGE, most
            # of it deferred past the gate window via unit thunks.
            for i in range(cc_n):
                nc.sync.dma_start(wk_sb[:, i, 0:128], wk_ext[i, :, 0:128])
                nc.sync.dma_start(mt_sb[:, i, 0:512], mt_ext[i, :, 0:512])
                nc.sync.dma_start(wq_sb[:, i, 0:128], wq_ext[i, :, 0:128])
            for i in range(cc_n // 2, cc_n):
                nc.sync.dma_start(xt_sb[:, i, 0:512], xt_ext[i, :, 0:512])
            for i in range(cc_n // 2):
                nc.scalar.dma_start(xt_sb[:, i, 0:512], xt_ext[i, :, 0:512])

            def bulk(eng, dst, src):
                eng.dma_start(dst, src.rearrange("c p f -> p c f"))

            bulk(nc.sync, mt_sb[:, :, 512:1024], mt_ext[:, :, 512:1024])
            bulk(nc.sync, wv_sb[:, :, :], wv_ext[:, :, :])
            bulk(nc.sync, mt_sb[:, :, 1024:1536], mt_ext[:, :, 1024:1536])
            bulk(nc.sync, mt_sb[:, :, 1536:2048], mt_ext[:, :, 1536:2048])
            bulk(nc.gpsimd, xt_sb[:, :, 512:1024], xt_ext[:, :, 512:1024])

            v_sb = qkv.tile([128, MC, HG, 64], BF16)
            kt_sb = qkv.tile([128, QC, M], BF16)       # 2-head pairs stacked
            qt_sb = qkv.tile([128, QC, N], BF16)

            # Projections emit as single 8-chunk bursts: the PE runs near
            # full clock and array-wide proj streams pipeline at ~216ns, so
            # fewer quadrant transitions beat shorter bursts.
            def _proj(w_ap_fn, mov_ap_fn, done_fn):
                ps = ps_proj.tile([128, 512], F32, tag="proj",
                                  name="proj_ps")
                n = len(ch)
                for j in range(n):
                    ci, rows = ch[j]
                    nc.tensor.matmul(
                        ps[:], w_ap_fn(ci, rows), mov_ap_fn(ci, rows),
                        start=(j == 0), stop=(j == n - 1),
                    )
                done_fn(ps)

            def proj_k(pair, mc):
                _proj(
                    lambda ci, rows: wk_sb[:rows, ci, pair * 128:(pair + 1) * 128],
                    lambda ci, rows: mt_sb[:rows, ci, mc * 512:(mc + 1) * 512],
                    lambda ps: nc.vector.tensor_copy(
                        kt_sb[:, pair, mc * 512:(mc + 1) * 512], ps[:]),
                )

            def proj_q(pair, qc):
                _proj(
                    lambda ci, rows: wq_sb[:rows, ci, pair * 128:(pair + 1) * 128],
                    lambda ci, rows: xt_sb[:rows, ci, qc * 512:(qc + 1) * 512],
                    lambda ps: nc.vector.tensor_copy(
                        qt_sb[:, pair, qc * 512:(qc + 1) * 512], ps[:]),
                )

            def proj_v(mt):
                _proj(
                    lambda ci, rows: mt_sb[:rows, ci, mt * 128:(mt + 1) * 128],
                    lambda ci, rows: wv_sb[:rows, ci, :],
                    lambda ps: nc.vector.tensor_copy(
                        v_sb[:, mt],
                        ps[:].rearrange("p (h d) -> p h d", h=HG)),
                )

            # per-block state: ship buffer + slot map, acc tiles, pt map
            blk = {}

            def s_exp(pair, qc, mt):
                """One unit: both heads' S matmuls into one PSUM tile
                (adjacent issue, disjoint PE row groups -> concurrent),
                then one exp on ScalarE (native) or DVE (Schraudolph)."""
                ps = ps_sp.tile([128, 1024], F32, tag="s")
                for h2 in range(2):
                    d0 = 64 * h2
                    nc.tensor.matmul(
                        ps[:, h2 * 512:(h2 + 1) * 512],
                        kt_sb[d0:d0 + 64, pair, mt * 128:(mt + 1) * 128],
                        qt_sb[d0:d0 + 64, pair, qc * 512:(qc + 1) * 512],
                        start=True, stop=True,
                    )
                pt_t = ptp.tile([128, 1024], BF16, tag="pt")
                pt_ap = pt_t[:]
                if mt in SCHRAU_MT:
                    nc.vector.tensor_scalar(
                        pt_ap.bitcast(I16), ps[:],
                        SCHRAU_A, SCHRAU_B, MULT, ADD)
                else:
                    nc.scalar.activation(pt_ap, ps[:], EXP)
                return pt_ap

            def pv(pair, mt, pt_ap, pso):
                for h2 in range(2):
                    head = 2 * pair + h2
                    nc.tensor.matmul(
                        pso[h2 * 64:(h2 + 1) * 64, :],
                        v_sb[:, mt, head, :],
                        pt_ap[:, h2 * 512:(h2 + 1) * 512],
                        start=(mt == 0), stop=(mt == MC - 1),
                    )

            def out_flush(pair, qc, pso):
                o_sb = osb.tile([128, 512], F32, tag="osb")
                nc.vector.tensor_copy(o_sb[:], pso[:])
                nc.sync.dma_start(out_ext[pair, qc], o_sb[:])

            # ---- emission schedule: one flat stream of 256 units ----
            # Unit u = (pair, qc, mt); exp fires per unit. Projection
            # work rides as per-unit thunks; PV matmuls drain from a FIFO
            # backlog once (a) their exp is PV_LAG units old and (b) for
            # pair 0 qc<=1, the V tile they need is emitted.
            PV_LAG = 3
            units = [(p, q, m) for p in range(QC) for q in range(QC)
                     for m in range(MC)]
            uidx = {u: i for i, u in enumerate(units)}

            sched = {}

            def at(u, fn):
                sched.setdefault(u, []).append(fn)

            at(1, lambda: proj_k(0, 1))
            at(3, lambda: proj_k(0, 2))
            at(5, lambda: proj_k(0, 3))
            at(2, lambda: bulk(nc.gpsimd, wk_sb[:, :, 128:512],
                               wk_ext[:, :, 128:512]))
            at(3, lambda: bulk(nc.gpsimd, xt_sb[:, :, 1024:1536],
                               xt_ext[:, :, 1024:1536]))
            at(4, lambda: bulk(nc.gpsimd, wq_sb[:, :, 128:512],
                               wq_ext[:, :, 128:512]))
            at(5, lambda: bulk(nc.gpsimd, xt_sb[:, :, 1536:2048],
                               xt_ext[:, :, 1536:2048]))
            v_unit = {m: 8 + 2 * m for m in range(MC)}
            for m in range(MC):
                at(v_unit[m], lambda mm=m: proj_v(mm))
            for p in range(QC):
                for q in range(QC):
                    if (p, q) == (0, 0):
                        continue
                    # (0,1)'s xt slice lands late: delay its Q projection
                    # so it doesn't block the PE FIFO.
                    prev = uidx[(p, q, 0)] - (4 if (p, q) == (0, 1) else 8)
                    at(prev, lambda pp=p, qq=q: proj_q(pp, qq))
            for p in range(QC - 1):
                # pair 0's K1 rides in (0,2) (V thunks occupy (0,1)'s
                # start); later pairs use their qc=1 block.
                base = uidx[(p, 2 if p == 0 else 1, 0)]
                for m in range(4):
                    at(base + 4 * m + 2,
                        lambda pp=p, mm=m: proj_k(pp + 1, mm))

            def v_ready(u, mt):
                return u >= v_unit[mt] + 2

            backlog = []           # (unit_emitted, (pair, qc, mt), pt_ap)
            cur = {"blk": None, "pso": None}

            def drain_one(u):
                eu, ent, pt_ap = backlog[0]
                p, q, mt = ent
                if u is not None and (
                        u < eu + PV_LAG
                        or (p == 0 and q <= 1 and not v_ready(u, mt))):
                    return False
                backlog.pop(0)
                if cur["blk"] != (p, q):
                    cur["blk"] = (p, q)
                    cur["pso"] = ps_op.tile([128, 512], F32, tag="o",
                                            name="pso")
                pv(p, mt, pt_ap, cur["pso"])
                if mt == MC - 1:
                    out_flush(p, q, cur["pso"])
                return True

            proj_k(0, 0)
            proj_q(0, 0)
            for u, (p, q, mt) in enumerate(units):
                if mt == 0:
                    blk.clear()
                    blk["pt"] = {}
                pt_ap = s_exp(p, q, mt)
                blk["pt"][mt] = pt_ap
                for fn in sched.get(u, ()):
                    fn()
                # --- accumulate roles ---
                if mt == DVE_ACC_MT[1]:
                    dacc = daccp.tile([128, 1024], BF16, tag="dacc",
                                      name="dacc_t")
                    blk["dacc"] = dacc
                    nc.vector.tensor_tensor(
                        dacc[:], blk["pt"][DVE_ACC_MT[0]],
                        blk["pt"][DVE_ACC_MT[1]], ADD)
                elif mt in DVE_ACC_MT[2:]:
                    dacc = blk["dacc"]
                    nc.vector.tensor_tensor(dacc[:], dacc[:], pt_ap, ADD)
                    if mt == DVE_ACC_MT[-1]:
                        nc.sync.dma_start(acc_ext[p, q, 0], dacc[:])
                elif mt == GP_MT[1]:
                    gacc1 = gaccp.tile([128, 1024], BF16, tag="gacc",
                                       name="gacc_t")
                    blk["gacc1"] = gacc1
                    nc.gpsimd.tensor_tensor(
                        gacc1[:], blk["pt"][GP_MT[0]],
                        blk["pt"][GP_MT[1]], ADD)
                    nc.sync.dma_start(acc_ext[p, q, 1], gacc1[:])
                elif mt == GP_MT[3]:
                    gacc2 = gaccp.tile([128, 1024], BF16, tag="gacc",
                                       name="gacc2_t")
                    blk["gacc2"] = gacc2
                    nc.gpsimd.tensor_tensor(
                        gacc2[:], blk["pt"][GP_MT[2]],
                        blk["pt"][GP_MT[3]], ADD)
                    nc.sync.dma_start(acc_ext[p, q, 2], gacc2[:])
                if mt in SHIP_MT:
                    nc.gpsimd.dma_start(
                        raw_ext[p, q, SHIP_MT.index(mt)], pt_ap)
                backlog.append((u, (p, q, mt), pt_ap))
                if u >= 200:
                    budget = 3 if len(backlog) > 2 else 1
                else:
                    budget = 3 if len(backlog) > 24 else (
                        2 if len(backlog) > 8 else 1)
                for _ in range(budget):
                    if not backlog or not drain_one(u):
                        break
            while backlog:
                drain_one(None)

    nc.compile()
    return nc


def _get_nc(with_bias: bool):
    if with_bias not in _NC_CACHE:
        _NC_CACHE[with_bias] = _build(with_bias)
    return _NC_CACHE[with_bias]


def kernel(input, memory, Wq, bq, Wk, bk, Wv, bv):
    input = np.asarray(input, np.float32)
    memory = np.asarray(memory, np.float32)
    scale = HEAD_DIM ** -0.5
    with_bias = bool(np.any(bq) or np.any(bk) or np.any(bv))
    nc = _get_nc(with_bias)

    bf = ml_dtypes.bfloat16

    def prep_act(x):
        # [N, DIM] -> [cc_n, 128, N] transposed chunks (+ ones row).
        xt = np.ascontiguousarray(x.T).reshape(CC, 128, x.shape[0])
        if with_bias:
            aug = np.zeros((1, 128, x.shape[0]), np.float32)
            aug[0, 0, :] = 1.0
            xt = np.concatenate([xt, aug], axis=0)
        return np.ascontiguousarray(xt.astype(bf))

    def prep_w(w, b, g, s=1.0):
        # [DIM, DIM] weight -> [cc_n, 128, COLS] of (W.T * s), head-group g.
        wt = (w.T[:, g * COLS:(g + 1) * COLS] * s).reshape(CC, 128, COLS)
        if with_bias:
            aug = np.zeros((1, 128, COLS), np.float32)
            aug[0, 0, :] = np.asarray(b, np.float32)[g * COLS:(g + 1) * COLS] * s
            wt = np.concatenate([wt, aug], axis=0)
        return np.ascontiguousarray(wt.astype(bf))

    in_maps = []
    for c in range(N_CORES):
        b_idx, g = divmod(c, 2)
        in_maps.append({
            "xt": prep_act(input[b_idx]),
            "mt": prep_act(memory[b_idx]),
            "wq": prep_w(np.asarray(Wq, np.float32), bq, g, scale),
            "wk": prep_w(np.asarray(Wk, np.float32), bk, g),
            "wv": prep_w(np.asarray(Wv, np.float32), bv, g),
        })

    kw = dict(_RUN_KWARGS)
    res = run_bass_kernel_spmd(nc, in_maps, list(range(N_CORES)), **kw)
    global LAST_RESULT
    LAST_RESULT = res

    out = np.empty((B, N, DIM), np.float32)
    for c in range(N_CORES):
        b_idx, g = divmod(c, 2)
        o = res.results[c]["out"]                    # [QC, QC, 128, 512]
        a = res.results[c]["accs"].astype(np.float32)  # [QC, QC, 3, 128, 1024]
        r = res.results[c]["raw"].astype(np.float32)   # [QC, QC, S, 128, 1024]
        sums = a.sum(axis=(2, 3)) + r[:, :, 0:N_SHIP].sum(axis=(2, 3))
        for p in range(QC):
            for qc in range(QC):
                blk = o[p, qc].reshape(2, 64, 512) / sums[p, qc].reshape(
                    2, 1, 512)
                out[b_idx, qc * 512:(qc + 1) * 512,
                    g * COLS + p * 128:g * COLS + (p + 1) * 128] = (
                    blk.transpose(2, 0, 1).reshape(512, 128))
    return out


# revision 11
# speedup vs baseline: 1.0253x; 1.0253x over previous
"""Multi-head cross-attention on 8 Trainium2 NeuronCores.

Problem (hardcoded): input [4, 2048, 1024], memory [4, 2048, 1024],
Wq/Wk/Wv [1024, 1024], bq/bk/bv [1024]; 16 heads x 64 dim; out
[4, 2048, 1024] f32.

Sharding: core c handles batch b = c//2 and head group g = c%2 (8
heads, output columns 512g:512g+512). Embarrassingly parallel - no
collectives.

Device dataflow (per core), everything contracted over channels with
host-pre-transposed operands so no on-chip transposes are needed:
  Q^T[d, n]  = sum_c WqT[c, d] * XT[c, n]     (depth scale folded in WqT)
  K^T[d, m]  = sum_c WkT[c, d] * MT[c, m]
  V[m, d]    = sum_c MT[c, m] * WvT[c, d]
  S^T[m, q]  = sum_d K^T[d, m] * Q^T[d, q]    (per head; heads of a pair
                                               row-packed: tile_position
                                               rows 0-63 / 64-127 run
                                               CONCURRENT on the PE)
  P^T        = exp(S^T)                        (no max subtraction;
                                               logits are O(5), safe)
  outU^T     = V^T P^T                         (per head; the two heads
                                               col-packed: tile_position
                                               cols 0-63 / 64-127 run
                                               CONCURRENT, own pt stream
                                               each -> full PE array)

ScalarE exp (256 ACTIVATEs of [128, 1024], ~1.03us effective each) is
the bottleneck, so the exp + softmax-denominator work is split across
engines per 16-unit (pair, qc) block:
  - mt in SCHRAU_MT: DVE computes exp via the Schraudolph bit trick in
    ONE tensor_scalar: int16(S * 184.66 + B) reinterpreted as bf16.
    ~2-3% elementwise error on those tiles only; the rest stay exact.
  - mt in SHIP_MT: the bf16 exp tile is shipped raw to DRAM (batched
    SWDGE DMA from a contiguous ship buffer); the HOST folds it into
    the softmax denominator. No on-device accumulate for those.
  - mt in GP_MT: GpSimd (idle otherwise) does the accumulate add.
  - remaining mt: DVE accumulates (first two tiles fused into one
    tensor_tensor).
Denominators: host sums the two per-block partial-acc ships (DVE's and
GpSimd's) plus the raw-shipped tiles over their 128 m-partitions, then
divides outU. Biases (zero here) are handled exactly via an extra K=1
contraction chunk when nonzero.

DMA plan: ScalarE's HWDGE ring carries ONLY the Q-projection gate
(per-chunk wq/xt so the first exp isn't delayed); the sync ring carries
the K gate per-chunk then the bulk inputs as few batched 3D DMAs; ALL
output traffic (out blocks, partial accs, raw pt ships) rides gpsimd
SWDGE. Queues fan their descriptors across all 16 DMA engines, so
batched DMAs cost the issuing engine ~1us regardless of size.

PSUM budget: S ping-pong 2x2 banks + proj 2 + pv 2 = 8.
"""

import numpy as np
import ml_dtypes

import concourse.bass as bass
import concourse.mybir as mybir
from concourse import bacc, tile
from concourse.bass_utils import run_bass_kernel_spmd

B, N, M, DIM = 4, 2048, 2048, 1024
NUM_HEADS, HEAD_DIM = 16, 64
HG = 8            # heads per core
COLS = HG * HEAD_DIM  # 512 output cols per core
N_CORES = 8
CC = DIM // 128   # 8 contraction chunks of 128
QC = 4            # q chunks of 512
MC = 16           # m (key) tiles of 128

F32 = mybir.dt.float32
BF16 = mybir.dt.bfloat16
I16 = mybir.dt.int16
EXP = mybir.ActivationFunctionType.Exp
ADD = mybir.AluOpType.add
MULT = mybir.AluOpType.mult

# Per-block (16 mt units) role assignment.
SCHRAU_MT = (2, 8, 14)        # DVE Schraudolph exp (subset of DVE acc set)
SHIP_MT = (1, 4, 7, 10, 13, 15)  # raw-shipped to host, no on-device accumulate
GP_MT = (3, 6, 9, 12)         # GpSimd accumulate pairs (3+6, 9+12)
DVE_ACC_MT = tuple(m for m in range(MC) if m not in SHIP_MT and m not in GP_MT)
N_SHIP = len(SHIP_MT)
SHIP_SLOTS = N_SHIP

# Schraudolph constants: bf16bits(exp(x)) ~= round(x*128*log2(e) + B).
SCHRAU_A = 128.0 * 1.4426950408889634
SCHRAU_B = 127.0 * 128.0 - 4.8

_NC_CACHE = {}
_RUN_KWARGS = {}   # test harness may inject trace=True etc.
LAST_RESULT = None


def _build(with_bias: bool):
    """Build the per-core SPMD Bass program."""
    cc_n = CC + (1 if with_bias else 0)
    nc = bacc.Bacc(None, target_bir_lowering=False)

    xt_ext = nc.declare_dram_parameter("xt", [cc_n, 128, N], BF16, isOutput=False)
    mt_ext = nc.declare_dram_parameter("mt", [cc_n, 128, M], BF16, isOutput=False)
    wq_ext = nc.declare_dram_parameter("wq", [cc_n, 128, COLS], BF16, isOutput=False)
    wk_ext = nc.declare_dram_parameter("wk", [cc_n, 128, COLS], BF16, isOutput=False)
    wv_ext = nc.declare_dram_parameter("wv", [cc_n, 128, COLS], BF16, isOutput=False)
    out_ext = nc.declare_dram_parameter("out", [QC, QC, 128, 512], F32, isOutput=True)
    acc_ext = nc.declare_dram_parameter(
        "accs", [QC, QC, 3, 128, 1024], BF16, isOutput=True)
    raw_ext = nc.declare_dram_parameter(
        "raw", [QC, QC, SHIP_SLOTS, 128, 1024], BF16, isOutput=True)

    ch = [(i, 128) for i in range(CC)]
    if with_bias:
        ch.append((CC, 1))

    with tile.TileContext(nc) as tc:
        with (
            tc.tile_pool(name="acts", bufs=1) as acts,
            tc.tile_pool(name="qkv", bufs=1) as qkv,
            tc.tile_pool(name="pt", bufs=26) as ptp,
            tc.tile_pool(name="daccp", bufs=2) as daccp,
            tc.tile_pool(name="gaccp", bufs=4) as gaccp,
            tc.tile_pool(name="osb", bufs=1) as osb,
            tc.tile_pool(name="ps_s", bufs=2, space="PSUM") as ps_sp,
            tc.tile_pool(name="ps_proj", bufs=2, space="PSUM") as ps_proj,
            tc.tile_pool(name="ps_o", bufs=2, space="PSUM") as ps_op,
        ):
            wk_sb = acts.tile([128, cc_n, COLS], BF16)
            wq_sb = acts.tile([128, cc_n, COLS], BF16)
            xt_sb = acts.tile([128, cc_n, N], BF16)
            wv_sb = acts.tile([128, cc_n, COLS], BF16)
            mt_sb = acts.tile([128, cc_n, M], BF16)

            # --- input DMAs ---
            # Gates per-chunk so projections overlap arrival: K gate on
            # the sync HWDGE ring, Q gate on the scalar ring (its ONLY
            # duty - ScalarE must spend the steady state on exp). Bulk
            # follows on sync as batched 3D DMAs (descriptors fan out
            # across all 16 DMA engines; issue cost ~600ns each).
            def bulk(eng, dst, src):
                eng.dma_start(dst, src.rearrange("c p f -> p c f"))

            # The K00+Q00 gate (wk/wq lo cols + mt/xt lo, 2.56MB) paces
            # the first exp. The HWDGE ring depth (~6 outstanding DMAs)
            # throttles per-chunk issue bursts to the transfer rate, so
            # each gate tensor ships as ONE batched 3D DMA; both rings
            # share the full HBM bandwidth and the gate lands in ~7us.
            # Exp-feeding bulk (mt) outranks wv (the PV backlog absorbs V
            # lateness); xt/w bulk rides SWDGE, mostly deferred past the
            # gate window via unit thunks.
            bulk(nc.sync, wk_sb[:, :, 0:128], wk_ext[:, :, 0:128])
            bulk(nc.sync, mt_sb[:, :, 0:512], mt_ext[:, :, 0:512])
            bulk(nc.sync, wq_sb[:, :, 0:128], wq_ext[:, :, 0:128])
            bulk(nc.scalar, xt_sb[:, :, 0:512], xt_ext[:, :, 0:512])

            bulk(nc.sync, mt_sb[:, :, 512:1024], mt_ext[:, :, 512:1024])
            bulk(nc.sync, wv_sb[:, :, :], wv_ext[:, :, :])
            bulk(nc.sync, mt_sb[:, :, 1024:1536], mt_ext[:, :, 1024:1536])
            bulk(nc.sync, mt_sb[:, :, 1536:2048], mt_ext[:, :, 1536:2048])
            bulk(nc.gpsimd, xt_sb[:, :, 512:1024], xt_ext[:, :, 512:1024])

            v_sb = qkv.tile([128, MC, HG, 64], BF16)
            kt_sb = qkv.tile([128, QC, M], BF16)       # 2-head pairs stacked
            qt_sb = qkv.tile([128, QC, N], BF16)

            # Projections emit as single 8-chunk bursts: the PE runs near
            # full clock and array-wide proj streams pipeline at ~216ns, so
            # fewer quadrant transitions beat shorter bursts.
            def _proj(w_ap_fn, mov_ap_fn, done_fn):
                ps = ps_proj.tile([128, 512], F32, tag="proj",
                                  name="proj_ps")
                n = len(ch)
                for j in range(n):
                    ci, rows = ch[j]
                    nc.tensor.matmul(
                        ps[:], w_ap_fn(ci, rows), mov_ap_fn(ci, rows),
                        start=(j == 0), stop=(j == n - 1),
                    )
                done_fn(ps)

            def proj_k(pair, mc):
                _proj(
                    lambda ci, rows: wk_sb[:rows, ci, pair * 128:(pair + 1) * 128],
                    lambda ci, rows: mt_sb[:rows, ci, mc * 512:(mc + 1) * 512],
                    lambda ps: nc.vector.tensor_copy(
                        kt_sb[:, pair, mc * 512:(mc + 1) * 512], ps[:]),
                )

            def proj_q(pair, qc):
                _proj(
                    lambda ci, rows: wq_sb[:rows, ci, pair * 128:(pair + 1) * 128],
                    lambda ci, rows: xt_sb[:rows, ci, qc * 512:(qc + 1) * 512],
                    lambda ps: nc.vector.tensor_copy(
                        qt_sb[:, pair, qc * 512:(qc + 1) * 512], ps[:]),
                )

            def proj_v(mt):
                _proj(
                    lambda ci, rows: mt_sb[:rows, ci, mt * 128:(mt + 1) * 128],
                    lambda ci, rows: wv_sb[:rows, ci, :],
                    lambda ps: nc.vector.tensor_copy(
                        v_sb[:, mt],
                        ps[:].rearrange("p (h d) -> p h d", h=HG)),
                )

            # per-block state: ship buffer + slot map, acc tiles, pt map
            blk = {}

            def s_exp(pair, qc, mt):
                """One unit: both heads' S matmuls into one PSUM tile
                (adjacent issue, disjoint PE row groups -> concurrent),
                then one exp on ScalarE (native) or DVE (Schraudolph)."""
                ps = ps_sp.tile([128, 1024], F32, tag="s")
                for h2 in range(2):
                    d0 = 64 * h2
                    nc.tensor.matmul(
                        ps[:, h2 * 512:(h2 + 1) * 512],
                        kt_sb[d0:d0 + 64, pair, mt * 128:(mt + 1) * 128],
                        qt_sb[d0:d0 + 64, pair, qc * 512:(qc + 1) * 512],
                        start=True, stop=True,
                    )
                pt_t = ptp.tile([128, 1024], BF16, tag="pt")
                pt_ap = pt_t[:]
                if mt in SCHRAU_MT:
                    nc.vector.tensor_scalar(
                        pt_ap.bitcast(I16), ps[:],
                        SCHRAU_A, SCHRAU_B, MULT, ADD)
                else:
                    nc.scalar.activation(pt_ap, ps[:], EXP)
                return pt_ap

            def pv(pair, mt, pt_ap, pso):
                for h2 in range(2):
                    head = 2 * pair + h2
                    nc.tensor.matmul(
                        pso[h2 * 64:(h2 + 1) * 64, :],
                        v_sb[:, mt, head, :],
                        pt_ap[:, h2 * 512:(h2 + 1) * 512],
                        start=(mt == 0), stop=(mt == MC - 1),
                    )

            def out_flush(pair, qc, pso):
                o_sb = osb.tile([128, 512], F32, tag="osb")
                nc.vector.tensor_copy(o_sb[:], pso[:])
                nc.sync.dma_start(out_ext[pair, qc], o_sb[:])

            # ---- emission schedule: one flat stream of 256 units ----
            # Unit u = (pair, qc, mt); exp fires per unit. Projection
            # work rides as per-unit thunks; PV matmuls drain from a FIFO
            # backlog once (a) their exp is PV_LAG units old and (b) for
            # pair 0 qc<=1, the V tile they need is emitted.
            PV_LAG = 3
            units = [(p, q, m) for p in range(QC) for q in range(QC)
                     for m in range(MC)]
            uidx = {u: i for i, u in enumerate(units)}

            sched = {}

            def at(u, fn):
                sched.setdefault(u, []).append(fn)

            at(1, lambda: proj_k(0, 1))
            at(3, lambda: proj_k(0, 2))
            at(5, lambda: proj_k(0, 3))
            at(2, lambda: bulk(nc.gpsimd, wk_sb[:, :, 128:512],
                               wk_ext[:, :, 128:512]))
            at(3, lambda: bulk(nc.gpsimd, xt_sb[:, :, 1024:1536],
                               xt_ext[:, :, 1024:1536]))
            at(4, lambda: bulk(nc.gpsimd, wq_sb[:, :, 128:512],
                               wq_ext[:, :, 128:512]))
            at(5, lambda: bulk(nc.gpsimd, xt_sb[:, :, 1536:2048],
                               xt_ext[:, :, 1536:2048]))
            v_unit = {m: 8 + 2 * m for m in range(MC)}
            for m in range(MC):
                at(v_unit[m], lambda mm=m: proj_v(mm))
            for p in range(QC):
                for q in range(QC):
                    if (p, q) == (0, 0):
                        continue
                    # (0,1)'s xt slice lands late: delay its Q projection
                    # so it doesn't block the PE FIFO.
                    prev = uidx[(p, q, 0)] - (4 if (p, q) == (0, 1) else 8)
                    at(prev, lambda pp=p, qq=q: proj_q(pp, qq))
            for p in range(QC - 1):
                # pair 0's K1 rides in (0,2) (V thunks occupy (0,1)'s
                # start); later pairs use their qc=1 block.
                base = uidx[(p, 2 if p == 0 else 1, 0)]
                for m in range(4):
                    at(base + 4 * m + 2,
                        lambda pp=p, mm=m: proj_k(pp + 1, mm))

            def v_ready(u, mt):
                return u >= v_unit[mt] + 2

            backlog = []           # (unit_emitted, (pair, qc, mt), pt_ap)
            cur = {"blk": None, "pso": None}

            def drain_one(u):
                eu, ent, pt_ap = backlog[0]
                p, q, mt = ent
                if u is not None and (
                        u < eu + PV_LAG
                        or (p == 0 and q <= 1 and not v_ready(u, mt))):
                    return False
                backlog.pop(0)
                if cur["blk"] != (p, q):
                    cur["blk"] = (p, q)
                    cur["pso"] = ps_op.tile([128, 512], F32, tag="o",
                                            name="pso")
                pv(p, mt, pt_ap, cur["pso"])
                if mt == MC - 1:
                    out_flush(p, q, cur["pso"])
                return True

            proj_k(0, 0)
            proj_q(0, 0)
            for u, (p, q, mt) in enumerate(units):
                if mt == 0:
                    blk.clear()
                    blk["pt"] = {}
                pt_ap = s_exp(p, q, mt)
                blk["pt"][mt] = pt_ap
                for fn in sched.get(u, ()):
                    fn()
                # --- accumulate roles ---
                if mt == DVE_ACC_MT[1]:
                    dacc = daccp.tile([128, 1024], BF16, tag="dacc",
                                      name="dacc_t")
                    blk["dacc"] = dacc
                    nc.vector.tensor_tensor(
                        dacc[:], blk["pt"][DVE_ACC_MT[0]],
                        blk["pt"][DVE_ACC_MT[1]], ADD)
                elif mt in DVE_ACC_MT[2:]:
                    dacc = blk["dacc"]
                    nc.vector.tensor_tensor(dacc[:], dacc[:], pt_ap, ADD)
                    if mt == DVE_ACC_MT[-1]:
                        nc.sync.dma_start(acc_ext[p, q, 0], dacc[:])
                elif mt == GP_MT[1]:
                    gacc1 = gaccp.tile([128, 1024], BF16, tag="gacc",
                                       name="gacc_t")
                    blk["gacc1"] = gacc1
                    nc.gpsimd.tensor_tensor(
                        gacc1[:], blk["pt"][GP_MT[0]],
                        blk["pt"][GP_MT[1]], ADD)
                    nc.sync.dma_start(acc_ext[p, q, 1], gacc1[:])
                elif mt == GP_MT[3]:
                    gacc2 = gaccp.tile([128, 1024], BF16, tag="gacc",
                                       name="gacc2_t")
                    blk["gacc2"] = gacc2
                    nc.gpsimd.tensor_tensor(
                        gacc2[:], blk["pt"][GP_MT[2]],
                        blk["pt"][GP_MT[3]], ADD)
                    nc.sync.dma_start(acc_ext[p, q, 2], gacc2[:])
                if mt in SHIP_MT:
                    nc.gpsimd.dma_start(
                        raw_ext[p, q, SHIP_MT.index(mt)], pt_ap)
                backlog.append((u, (p, q, mt), pt_ap))
                if u >= 200:
                    budget = 3 if len(backlog) > 2 else 1
                else:
                    budget = 3 if len(backlog) > 24 else (
                        2 if len(backlog) > 8 else 1)
                for _ in range(budget):
                    if not backlog or not drain_one(u):
                        break
            while backlog:
                drain_one(None)

    nc.compile()
    return nc


def _get_nc(with_bias: bool):
    if with_bias not in _NC_CACHE:
        _NC_CACHE[with_bias] = _build(with_bias)
    return _NC_CACHE[with_bias]


def kernel(input, memory, Wq, bq, Wk, bk, Wv, bv):
    input = np.asarray(input, np.float32)
    memory = np.asarray(memory, np.float32)
    scale = HEAD_DIM ** -0.5
    with_bias = bool(np.any(bq) or np.any(bk) or np.any(bv))
    nc = _get_nc(with_bias)

    bf = ml_dtypes.bfloat16

    def prep_act(x):
        # [N, DIM] -> [cc_n, 128, N] transposed chunks (+ ones row).
        xt = np.ascontiguousarray(x.T).reshape(CC, 128, x.shape[0])
        if with_bias:
            aug = np.zeros((1, 128, x.shape[0]), np.float32)
            aug[0, 0, :] = 1.0
            xt = np.concatenate([xt, aug], axis=0)
        return np.ascontiguousarray(xt.astype(bf))

    def prep_w(w, b, g, s=1.0):
        # [DIM, DIM] weight -> [cc_n, 128, COLS] of (W.T * s), head-group g.
        wt = (w.T[:, g * COLS:(g + 1) * COLS] * s).reshape(CC, 128, COLS)
        if with_bias:
            aug = np.zeros((1, 128, COLS), np.float32)
            aug[0, 0, :] = np.asarray(b, np.float32)[g * COLS:(g + 1) * COLS] * s
            wt = np.concatenate([wt, aug], axis=0)
        return np.ascontiguousarray(wt.astype(bf))

    in_maps = []
    for c in range(N_CORES):
        b_idx, g = divmod(c, 2)
        in_maps.append({
            "xt": prep_act(input[b_idx]),
            "mt": prep_act(memory[b_idx]),
            "wq": prep_w(np.asarray(Wq, np.float32), bq, g, scale),
            "wk": prep_w(np.asarray(Wk, np.float32), bk, g),
            "wv": prep_w(np.asarray(Wv, np.float32), bv, g),
        })

    kw = dict(_RUN_KWARGS)
    res = run_bass_kernel_spmd(nc, in_maps, list(range(N_CORES)), **kw)
    global LAST_RESULT
    LAST_RESULT = res

    out = np.empty((B, N, DIM), np.float32)
    for c in range(N_CORES):
        b_idx, g = divmod(c, 2)
        o = res.results[c]["out"]                    # [QC, QC, 128, 512]
        a = res.results[c]["accs"].astype(np.float32)  # [QC, QC, 3, 128, 1024]
        r = res.results[c]["raw"].astype(np.float32)   # [QC, QC, S, 128, 1024]
        sums = a.sum(axis=(2, 3)) + r[:, :, 0:N_SHIP].sum(axis=(2, 3))
        for p in range(QC):
            for qc in range(QC):
                blk = o[p, qc].reshape(2, 64, 512) / sums[p, qc].reshape(
                    2, 1, 512)
                out[b_idx, qc * 512:(qc + 1) * 512,
                    g * COLS + p * 128:g * COLS + (p + 1) * 128] = (
                    blk.transpose(2, 0, 1).reshape(512, 128))
    return out
